# revision 6
# baseline (speedup 1.0000x reference)
"""Causal single-head attention (B=4, S=4096, D=2048) on 8 trn2 NeuronCores.

Sharding: core = (batch b, query-half h). Query blocks of 128 rows are
interleaved between the two halves ({4j,4j+3} vs {4j+1,4j+2} within each
group of 4) so that both halves execute an identical instruction stream
(SPMD) with balanced causal work. Per core: 8 strips of 256 queries;
strip j attends to keys [0, 512*(j+1)).

v2 structure:
  - x arrives pre-transposed from the host (d-major), so every load is a
    plain contiguous DMA (no on-device transposes).
  - Projection order K -> V -> Q; the pair AllGathers of K^T and V
    overlap the following projection phases. SBUF slots are recycled by
    tag (wk->wq, wv->xTq, xTo->Q^T) so each successor's loads prefetch
    as soon as the predecessor's last matmul has read the slot.
  - Q^T stays resident in SBUF through attention (no DRAM round-trip).
  - Softmax denominators are computed inside the qp=0 AV pass, reusing
    the AV matmuls' stationary P^T chunks (no extra LDWEIGHTS): all four
    query-subblocks accumulate into one PSUM bank, a single start=True
    clears it, and every other denominator matmul relies on the
    per-element has_written overwrite-on-first-touch behaviour.
All matmuls run in bf16 (fp32 PSUM accumulation); softmax is computed
without max-subtraction (scores are O(1) here) in the transposed layout
scores^T = K.Q so no on-chip transposes are needed anywhere.
"""

import sys

try:
    import concourse  # noqa: F401
except ImportError:
    sys.path.insert(0, "/opt/trn_rl_repo")

import numpy as np
import ml_dtypes

import concourse.bass as bass
import concourse.mybir as mybir
import concourse.tile as tile
from concourse import bacc
from concourse.bass_utils import run_bass_kernel_spmd

B, S, D = 4, 4096, 2048
NQ = S // 2          # queries per core
C = D // 128         # 16 contraction chunks
STRIPS = 8           # strips of 256 queries per core
SQ = NQ // STRIPS    # 256
SCALE = 1.0 / float(np.sqrt(D))

BF = mybir.dt.bfloat16
F32 = mybir.dt.float32


def _blocks_for_half(h: int) -> list[int]:
    # strip-major order; strip j covers global blocks {4j+0,4j+3} or {4j+1,4j+2}
    off = (0, 3) if h == 0 else (1, 2)
    return [4 * j + o for j in range(STRIPS) for o in off]


def build_nc():
    nc = bacc.Bacc("TRN2", target_bir_lowering=False, debug=False, num_devices=8)

    xTo = nc.dram_tensor("xTo", [D, NQ], BF, kind="ExternalInput")
    xTq = nc.dram_tensor("xTq", [D, NQ], BF, kind="ExternalInput")
    wq = nc.dram_tensor("Wq", [D, D], BF, kind="ExternalInput")
    wk = nc.dram_tensor("Wk", [D, D], BF, kind="ExternalInput")
    wv = nc.dram_tensor("Wv", [D, D], BF, kind="ExternalInput")
    # maskT[512*j + kk, qq]: multiplicative mask for strip j's diagonal key
    # group, key-major (matches the transposed score layout)
    maskT = nc.dram_tensor("maskT", [S, SQ], BF, kind="ExternalInput")
    out = nc.dram_tensor("out", [NQ, D], F32, kind="ExternalOutput")

    # d-major tiled: [d-chunk, d-in-chunk, seq]. Each core projects only its
    # own half of the keys; pair AllGather fills the rank-outer full tensors.
    kT_half = nc.dram_tensor("kT_half", [C, 128, NQ], BF, kind="Internal")
    kT = nc.dram_tensor("kT", [2, C, 128, NQ], BF, kind="Internal")
    vN_half = nc.dram_tensor("vN_half", [NQ, D], BF, kind="Internal")
    vN = nc.dram_tensor("vN", [2, NQ, D], BF, kind="Internal")

    with tile.TileContext(nc) as tc:
        _emit(nc, tc, xTo, xTq, wq, wk, wv, maskT, out, kT_half, kT, vN_half, vN)

    nc.compile()
    return nc


def _emit(nc, tc, xTo, xTq, wq, wk, wv, maskT, out, kT_half, kT, vN_half, vN):
    PAIRS = [[0, 1], [2, 3], [4, 5], [6, 7]]

    # xt pool outlives the projections: its slots are recycled (by tag) to
    # hold Q^T through the attention phase.
    with tc.tile_pool(name="xt", bufs=1) as xtp:
        with (
            tc.tile_pool(name="w", bufs=1) as wp,
            tc.tile_pool(name="wv", bufs=1) as wvp,
        ):
            # Load order feeds the K matmul ramp: the first m-block of chains
            # needs wk[:, 0:512] and xTo quarter 0 only, so those land first.
            wk_sb, wv_sb, xto_sb = [], [], []
            for c in range(C):
                wk_sb.append(wp.tile([128, D], BF, name=f"w{c}"))
                xto_sb.append(xtp.tile([128, NQ], BF, name=f"xt{c}"))
            for c in range(C):
                nc.sync.dma_start(
                    out=wk_sb[c][:, 0:512], in_=wk.ap()[128 * c : 128 * (c + 1), 0:512]
                )
                nc.sync.dma_start(
                    out=xto_sb[c][:, 0:512],
                    in_=xTo.ap()[128 * c : 128 * (c + 1), 0:512],
                )
            for q4 in range(1, 4):
                for c in range(C):
                    nc.sync.dma_start(
                        out=xto_sb[c][:, 512 * q4 : 512 * (q4 + 1)],
                        in_=xTo.ap()[128 * c : 128 * (c + 1), 512 * q4 : 512 * (q4 + 1)],
                    )
            for c in range(C):
                nc.sync.dma_start(
                    out=wk_sb[c][:, 512:D], in_=wk.ap()[128 * c : 128 * (c + 1), 512:D]
                )
            for c in range(C):
                t = wvp.tile([128, D], BF, name=f"wv{c}")
                nc.scalar.dma_start(out=t[:], in_=wv.ap()[128 * c : 128 * (c + 1), :])
                wv_sb.append(t)

            # One PSUM/copy pool pair spans all three projections (no pool
            # boundary gaps between phases).
            with (
                tc.tile_pool(name="pps", bufs=8, space="PSUM") as pps,
                tc.tile_pool(name="pcp", bufs=8) as pcp,
            ):
                # ---- K^T projection: kT_half[m, :, s] d-major ----
                for m in range(C):
                    ps = [pps.tile([128, 512], F32, name="pps_t") for _ in range(4)]
                    for c in range(C):
                        for s4 in range(4):
                            nc.tensor.matmul(
                                ps[s4][:], lhsT=wk_sb[c][:, 128 * m : 128 * (m + 1)],
                                rhs=xto_sb[c][:, 512 * s4 : 512 * (s4 + 1)],
                                start=(c == 0), stop=(c == C - 1),
                            )
                    for s4 in range(4):
                        o = pcp.tile([128, 512], BF, name="pcp_t")
                        nc.scalar.copy(o[:], ps[s4][:])
                        nc.scalar.dma_start(
                            out=kT_half.ap()[m, :, 512 * s4 : 512 * (s4 + 1)], in_=o[:]
                        )

                nc.gpsimd.collective_compute(
                    "AllGather", mybir.AluOpType.bypass, replica_groups=PAIRS,
                    ins=[kT_half.ap().opt()], outs=[kT.ap().opt()],
                )

                # Wq prefetch into the wk slots (frees as K's matmuls finish)
                wq_sb = []
                for c in range(C):
                    t = wp.tile([128, D], BF, name=f"w{c}")
                    nc.sync.dma_start(
                        out=t[:], in_=wq.ap()[128 * c : 128 * (c + 1), :]
                    )
                    wq_sb.append(t)

                # ---- V projection (natural layout): xTo stationary, Wv moving
                for u in range(C):
                    ps = [pps.tile([128, 512], F32, name="pps_t") for _ in range(4)]
                    for c in range(C):
                        for n4 in range(4):
                            nc.tensor.matmul(
                                ps[n4][:],
                                lhsT=xto_sb[c][:, 128 * u : 128 * (u + 1)],
                                rhs=wv_sb[c][:, 512 * n4 : 512 * (n4 + 1)],
                                start=(c == 0), stop=(c == C - 1),
                            )
                    for n4 in range(4):
                        o = pcp.tile([128, 512], BF, name="pcp_t")
                        nc.scalar.copy(o[:], ps[n4][:])
                        nc.scalar.dma_start(
                            out=vN_half.ap()[
                                128 * u : 128 * (u + 1), 512 * n4 : 512 * (n4 + 1)
                            ],
                            in_=o[:],
                        )

                nc.gpsimd.collective_compute(
                    "AllGather", mybir.AluOpType.bypass, replica_groups=PAIRS,
                    ins=[vN_half.ap().opt()], outs=[vN.ap().opt()],
                )

                # ---- Q^T projection into SBUF-resident tiles ----
                # xTq recycles the wv slots (quarter 0 first to shorten the
                # post-V bubble); Q^T recycles the xTo slots.
                xtq_sb, qt_sb = [], []
                for c in range(C):
                    xtq_sb.append(wvp.tile([128, NQ], BF, name=f"wv{c}"))
                    qt_sb.append(xtp.tile([128, NQ], BF, name=f"xt{c}"))
                for q4 in range(4):
                    for c in range(C):
                        nc.sync.dma_start(
                            out=xtq_sb[c][:, 512 * q4 : 512 * (q4 + 1)],
                            in_=xTq.ap()[
                                128 * c : 128 * (c + 1), 512 * q4 : 512 * (q4 + 1)
                            ],
                        )
                for m in range(C):
                    ps = [pps.tile([128, 512], F32, name="pps_t") for _ in range(4)]
                    for c in range(C):
                        for s4 in range(4):
                            nc.tensor.matmul(
                                ps[s4][:],
                                lhsT=wq_sb[c][:, 128 * m : 128 * (m + 1)],
                                rhs=xtq_sb[c][:, 512 * s4 : 512 * (s4 + 1)],
                                start=(c == 0), stop=(c == C - 1),
                            )
                    for s4 in range(4):
                        nc.scalar.copy(
                            qt_sb[m][:, 512 * s4 : 512 * (s4 + 1)], ps[s4][:]
                        )

        # w/wv pools released; attention pools take their space.
        _attention(nc, tc, qt_sb, kT, vN, maskT, out)


def _attention(nc, tc, qt_sb, kT, vN, maskT, out):
    # Pair p covers strips 2p (queries [512p, 512p+256), key bound 512(2p+1))
    # and 2p+1 (queries [512p+256, 512p+512), bound 512(2p+2)). Scores run
    # pair-wide (N=512) except the last key group (odd member only, N=256);
    # AV runs in 4 d-quarter passes so 4 query-sub PSUM tiles fit in 4 banks.
    with (
        tc.tile_pool(name="ones", bufs=1) as onesp,
        tc.tile_pool(name="kg", bufs=32) as kgp,
        tc.tile_pool(name="pt", bufs=48) as ptp,
        tc.tile_pool(name="vg", bufs=16) as vgp,
        tc.tile_pool(name="msk", bufs=8) as mskp,
        tc.tile_pool(name="rcp", bufs=8) as rcpp,
        tc.tile_pool(name="osb", bufs=6) as osbp,
        tc.tile_pool(name="ps_s", bufs=2, space="PSUM") as ps_s,
        tc.tile_pool(name="ps_o", bufs=5, space="PSUM") as ps_o,
        tc.tile_pool(name="ps_d", bufs=1, space="PSUM") as ps_d,
    ):
        ones = onesp.tile([128, 1], BF)
        nc.vector.memset(ones[:], 1.0)

        NPAIR = STRIPS // 2
        for i in range(NPAIR):
            ng_even = 2 * i + 1   # groups for subs 0,1 (strip 2i)
            ng_odd = 2 * i + 2    # groups for subs 2,3 (strip 2i+1)

            # Phase A: P^T chunks. Column-start trim per chunk: the first
            # 128-query block of each strip is fully masked (for both query
            # halves) against the upper kk sub-chunks of its diagonal key
            # group, so those columns are skipped outright.
            def col_start(g, kk):
                if g == 2 * i + 1:  # member1-only group (= member1's diagonal)
                    return 256 if kk < 2 else 384
                if g == 2 * i and kk >= 2:  # member0's diagonal, block A dead
                    return 128
                return 0

            pt = []
            for g in range(ng_odd):
                kg = []
                for c in range(C):
                    t = kgp.tile([128, 512], BF, name="kg_t")
                    gl = 512 * (g % 4)
                    nc.sync.dma_start(
                        out=t[:], in_=kT.ap()[g // 4, c, :, gl : gl + 512]
                    )
                    kg.append(t)
                for kk in range(4):
                    cs = col_start(g, kk)
                    ps = ps_s.tile([128, 512], F32, name="ps_s_t")
                    for c in range(C):
                        nc.tensor.matmul(
                            ps[:, cs:512],
                            lhsT=kg[c][:, 128 * kk : 128 * (kk + 1)],
                            rhs=qt_sb[c][:, 512 * i + cs : 512 * (i + 1)],
                            start=(c == 0), stop=(c == C - 1),
                        )
                    p = ptp.tile([128, 512], BF, name="pt_t")
                    nc.scalar.activation(
                        out=p[:, cs:512], in_=ps[:, cs:512],
                        func=mybir.ActivationFunctionType.Exp, scale=SCALE,
                    )
                    # diagonal-group masks, per member strip
                    for member, js in ((0, 2 * i), (1, 2 * i + 1)):
                        if g == js:
                            mk = mskp.tile([128, SQ], BF, name="msk_t")
                            r0 = 512 * js + 128 * kk
                            nc.sync.dma_start(
                                out=mk[:], in_=maskT.ap()[r0 : r0 + 128, :]
                            )
                            if kk < 2:
                                cols = slice(256 * member, 256 * member + 256)
                                mcols = slice(0, 256)
                            else:  # block A columns were skipped
                                cols = slice(256 * member + 128, 256 * member + 256)
                                mcols = slice(128, 256)
                            nc.vector.tensor_mul(
                                p[:, cols], p[:, cols], mk[:, mcols]
                            )
                    pt.append(p)

            # Phase B: 4 d-quarter AV passes. The qp=0 pass also accumulates
            # softmax denominators into one PSUM bank (den[:, u]), reusing
            # each AV matmul's stationary P^T chunk so no extra LDWEIGHTS is
            # paid. A single start=True clears the bank; every other den
            # matmul relies on per-element overwrite-on-first-touch.
            def av_included(u, g, kk):
                ng_u = ng_even if u < 2 else ng_odd
                if g >= ng_u:
                    return False
                # chunks whose P^T columns were never computed (trimmed)
                if u == 0 and g == 2 * i and kk >= 2:
                    return False
                if u == 2 and g == 2 * i + 1 and kk >= 2:
                    return False
                return True

            av_last = {
                0: (2 * i, 1), 1: (2 * i, 3),
                2: (2 * i + 1, 1), 3: (2 * i + 1, 3),
            }
            den = ps_d.tile([128, 4], F32, name="den_t")
            rec = None
            for qp in range(4):
                o_ps = [
                    ps_o.tile([128, 512], F32, name="o_ps") for _ in range(4)
                ]
                for g in range(ng_odd):
                    for kk in range(4):
                        kc = 4 * g + kk
                        r0 = 512 * g + 128 * kk
                        vt = vgp.tile([128, 512], BF, name="vg_t")
                        rl = r0 % NQ
                        nc.scalar.dma_start(
                            out=vt[:],
                            in_=vN.ap()[r0 // NQ, rl : rl + 128, 512 * qp : 512 * (qp + 1)],
                        )
                        for u in range(4):
                            if not av_included(u, g, kk):
                                continue
                            lh = pt[kc][:, 128 * u : 128 * (u + 1)]
                            first = g == 0 and kk == 0
                            last = (g, kk) == av_last[u]
                            nc.tensor.matmul(
                                o_ps[u][:], lhsT=lh, rhs=vt[:],
                                start=first, stop=last,
                            )
                            if qp == 0:
                                nc.tensor.matmul(
                                    den[:, u : u + 1], lhsT=lh, rhs=ones[:],
                                    start=(first and u == 0),
                                    stop=(g == ng_odd - 1 and kk == 3 and u == 3),
                                    skip_group_check=True,
                                )
                if qp == 0:
                    rec = rcpp.tile([128, 4], F32, name="rec_t")
                    nc.vector.reciprocal(rec[:], den[:])
                for u in range(4):
                    o = osbp.tile([128, 512], F32, name="osb_t")
                    nc.vector.tensor_scalar_mul(o[:], o_ps[u][:], rec[:, u : u + 1])
                    r0 = 512 * i + 128 * u
                    nc.sync.dma_start(
                        out=out.ap()[r0 : r0 + 128, 512 * qp : 512 * (qp + 1)],
                        in_=o[:],
                    )


_NC_CACHE = None


def _get_nc():
    global _NC_CACHE
    if _NC_CACHE is None:
        _NC_CACHE = build_nc()
    return _NC_CACHE


def _core_inputs(x, Wq, Wk, Wv, b, h):
    blocks = _blocks_for_half(h)
    qpos = (128 * np.asarray(blocks)[:, None] + np.arange(128)[None, :]).reshape(-1)
    xb = np.ascontiguousarray(x[b])
    maskT = np.zeros((S, SQ), dtype=np.float32)
    for j in range(STRIPS):
        keys = 512 * j + np.arange(512)[:, None]
        qp = qpos[SQ * j : SQ * (j + 1)][None, :]
        maskT[512 * j : 512 * (j + 1), :] = (keys <= qp).astype(np.float32)
    x_own = xb[NQ * h : NQ * (h + 1)]
    return {
        "xTq": np.ascontiguousarray(xb[qpos].T).astype(ml_dtypes.bfloat16),
        "xTo": np.ascontiguousarray(x_own.T).astype(ml_dtypes.bfloat16),
        "Wq": np.ascontiguousarray(Wq).astype(ml_dtypes.bfloat16),
        "Wk": np.ascontiguousarray(Wk).astype(ml_dtypes.bfloat16),
        "Wv": np.ascontiguousarray(Wv).astype(ml_dtypes.bfloat16),
        "maskT": maskT.astype(ml_dtypes.bfloat16),
    }, qpos


def kernel(x, Wq, Wk, Wv, _want_results=False):
    x = np.asarray(x)
    Wq, Wk, Wv = np.asarray(Wq), np.asarray(Wk), np.asarray(Wv)
    nc = _get_nc()

    in_maps, qposes = [], []
    for b in range(B):
        for h in range(2):
            im, qpos = _core_inputs(x, Wq, Wk, Wv, b, h)
            in_maps.append(im)
            qposes.append((b, qpos))

    res = run_bass_kernel_spmd(nc, in_maps, core_ids=list(range(8)))

    out = np.empty((B, S, D), dtype=np.float32)
    for core, (b, qpos) in enumerate(qposes):
        out[b][qpos] = res.results[core]["out"]
    if _want_results:
        return out, res
    return out


def measure_exec_ns(inputs, iters=48):
    """Estimate per-launch device execution time by pipelining `iters`
    dispatches of the compiled executable with device-resident inputs
    (amortizes host/tunnel dispatch overhead); returns marginal ns/exec."""
    import time
    import jax
    from jax.sharding import Mesh, PartitionSpec, NamedSharding
    from jax.experimental.shard_map import shard_map
    from concourse.bass2jax import (
        _bass_exec_p, install_neuronx_cc_hook, partition_id_tensor,
    )

    nc = _get_nc()
    install_neuronx_cc_hook()
    in_names, out_names, out_avals, zero_outs = [], [], [], []
    for alloc in nc.m.functions[0].allocations:
        if not isinstance(alloc, mybir.MemoryLocationSet):
            continue
        name = alloc.memorylocations[0].name
        if alloc.kind == "ExternalInput":
            if nc.partition_id_tensor is None or name != nc.partition_id_tensor.name:
                in_names.append(name)
        elif alloc.kind == "ExternalOutput":
            out_names.append(name)
            shape = tuple(alloc.tensor_shape)
            dtype = mybir.dt.np(alloc.dtype)
            out_avals.append(jax.core.ShapedArray(shape, dtype))
            zero_outs.append(np.zeros(shape, dtype))
    n_params = len(in_names)
    n_outs = len(out_avals)
    all_names = in_names + out_names
    if nc.partition_id_tensor is not None:
        all_names = all_names + [nc.partition_id_tensor.name]

    def _body(*args):
        operands = list(args)
        if nc.partition_id_tensor is not None:
            operands.append(partition_id_tensor())
        return tuple(_bass_exec_p.bind(
            *operands, out_avals=tuple(out_avals), in_names=tuple(all_names),
            out_names=tuple(out_names), lowering_input_output_aliases=(),
            sim_require_finite=True, sim_require_nnan=True, nc=nc,
        ))

    devices = jax.devices()[:8]
    mesh = Mesh(np.array(devices), ("core",))
    sharded = jax.jit(
        shard_map(_body, mesh=mesh,
                  in_specs=(PartitionSpec("core"),) * (n_params + n_outs),
                  out_specs=(PartitionSpec("core"),) * n_outs,
                  check_rep=False),
        donate_argnums=tuple(range(n_params, n_params + n_outs)),
        keep_unused=True,
    )
    in_maps = []
    x, Wq, Wk, Wv = inputs["x"], inputs["Wq"], inputs["Wk"], inputs["Wv"]
    for b in range(B):
        for h in range(2):
            im, _ = _core_inputs(x, Wq, Wk, Wv, b, h)
            in_maps.append(im)
    sh = NamedSharding(mesh, PartitionSpec("core"))
    concat_in = [
        jax.device_put(
            np.concatenate([np.asarray(in_maps[c][n]) for c in range(8)], axis=0), sh
        )
        for n in in_names
    ]

    def put_zeros():
        return [
            jax.device_put(np.zeros((8 * z.shape[0], *z.shape[1:]), z.dtype), sh)
            for z in zero_outs
        ]

    jax.block_until_ready(sharded(*concat_in, *put_zeros()))  # warmup
    times = {}
    for K in (4, iters, 4, iters):
        zs = [put_zeros() for _ in range(K)]
        jax.block_until_ready(zs)
        t0 = time.time()
        outs = [sharded(*concat_in, *z) for z in zs]
        jax.block_until_ready(outs)
        times[K] = min(times.get(K, 1e9), time.time() - t0)
    slope = (times[iters] - times[4]) / (iters - 4)
    return int(slope * 1e9)
